# revision 30
# baseline (speedup 1.0000x reference)
"""Trainium2 Bass kernel for ExpMemoryUpdater (scatter_memory).

Semantics (reference):
    mem_rows   = memory[unique_node_ids]                  # [n_upd, dim]
    decay      = exp((last_update[unique_node_ids] - timestamps) / LAMB)
    updated    = unique_messages + decay[:, None] * mem_rows
    updated_memory  = memory.at[unique_node_ids].set(updated)
    new_last_update = last_update.at[unique_node_ids].set(timestamps)

Distribution: the 100000 updated rows are row-sharded across 8 NeuronCores
(12500 rows each, padded to 12544 = 128*98). Each core computes its updated
rows; rows not touched by any update and the last_update scatter are pure
data placement, handled during host-side unshard/assembly.

On-core layout: partition p owns 98 consecutive rows of its shard, so every
DMA moves long contiguous per-partition runs (up to 14 KB per partition per
chunk, 1.75 MB per dma_start; ~368 GB/s effective, at the per-core HBM
roofline). decay = exp((lu-ts)/LAMB) is computed once as a [128, 98] tile
with a ~2 ulp software exp (Cody-Waite + Taylor; the ACT table Exp is only
~1e-5 accurate). The row update is two whole-chunk DVE ops, with decay
broadcast along dim via a 0-stride AP:
    mem *= decay ; out = mem + msg
Loads ride the SP HWDGE ring and stores the ACT ring, so a store waiting on
compute never head-of-line-blocks later loads. Final chunks shrink so the
compute+store tail past the last load is short. Measured ~104 us HW exec
vs a ~100 us DMA roofline (36.75 MB/core at 358 GB/s).
"""

import sys
import types

import numpy as np

N_CORES = 8
N_NODES = 200000
DIM = 256
N_UPD = 100000
LAMB = 10.0

P = 128          # SBUF partitions
NBLK = 98        # row-blocks per partition
RPC = P * NBLK   # rows per core, padded (12544)
# Row-blocks per DMA chunk. Large chunks amortize DMA setup; the final
# chunks shrink so the compute+store tail past the last load is short.
CHUNKS = [16, 16, 16, 16, 16, 10, 4, 2, 1, 1]
assert sum(CHUNKS) == NBLK
CH_MAX = max(CHUNKS)


def _install_ntff_hook_shim():
    """Make run_bass_kernel_spmd(trace=True) work under axon: the stock
    antenv package lacks axon_hooks, so register the ctypes NTFF hook from
    trn_agent_boot under that name."""
    try:
        import antenv.axon_hooks  # noqa: F401
        return
    except ImportError:
        pass
    try:
        import antenv
        from trn_agent_boot.trn_boot import _ntff_profile_via_ctypes

        mod = types.ModuleType("antenv.axon_hooks")
        hook = [_ntff_profile_via_ctypes("/opt/axon/libaxon_pjrt.so")]
        mod.set_axon_ntff_profile_hook = lambda h: hook.__setitem__(0, h)
        mod.get_axon_ntff_profile_hook = lambda: hook[0]
        sys.modules["antenv.axon_hooks"] = mod
        antenv.axon_hooks = mod
    except Exception:
        pass


_NC_CACHE = {}


def _emit_decay_chain(ve, mybir, tiles, lu_ap, ts_ap):
    """Emit the software-exp op chain on the vector engine `ve`.

    tiles: dict with f32 [P, NBLK] tiles lu,ts,x,y,k,u,q,decay and int32 sc.
    Computes decay = exp((lu - ts)/LAMB) to ~2 ulp.
    """
    import math

    sub = mybir.AluOpType.subtract
    mult = mybir.AluOpType.mult
    add = mybir.AluOpType.add
    shl = mybir.AluOpType.arith_shift_left
    dt = mybir.dt.float32

    INV_LAMB = float(np.float32(1.0 / LAMB))
    LOG2E = 1.4426950408889634
    MAGIC = 12582912.0  # 1.5 * 2^23
    LN2_HI = 0.693145751953125  # 0x3F317200, 9 trailing zero bits
    LN2_LO = 0.6931471805599453 - LN2_HI
    POLY = [1.0 / math.factorial(i) for i in range(8)]  # c0..c7

    t = tiles
    ve.tensor_tensor(t["x"][:], lu_ap, ts_ap, sub)
    ve.tensor_scalar_mul(t["x"][:], t["x"][:], INV_LAMB)
    # y = x*log2e + MAGIC;  k = y - MAGIC  (round-to-nearest)
    ve.tensor_scalar(
        out=t["y"][:], in0=t["x"][:], scalar1=LOG2E, scalar2=MAGIC,
        op0=mult, op1=add,
    )
    ve.tensor_scalar_sub(t["k"][:], t["y"][:], MAGIC)
    # u = (k*(-ln2_hi) + x) + k*(-ln2_lo)
    ve.scalar_tensor_tensor(
        out=t["u"][:], in0=t["k"][:], scalar=-LN2_HI, in1=t["x"][:],
        op0=mult, op1=add,
    )
    ve.scalar_tensor_tensor(
        out=t["u"][:], in0=t["k"][:], scalar=-LN2_LO, in1=t["u"][:],
        op0=mult, op1=add,
    )
    # exp(u) = c0 + u*(c1 + u*(... + u*c7)) via q = (q + c)*u steps
    ve.tensor_scalar_mul(t["q"][:], t["u"][:], POLY[7])
    for c_i in POLY[6:0:-1]:
        ve.scalar_tensor_tensor(
            out=t["q"][:], in0=t["q"][:], scalar=float(c_i), in1=t["u"][:],
            op0=add, op1=mult,
        )
    ve.tensor_scalar_add(t["q"][:], t["q"][:], 1.0)
    # bits(2^k) = (bits(y) << 23) + 0x3F800000
    ve.tensor_scalar(
        out=t["sc"][:], in0=t["y"][:].bitcast(mybir.dt.int32),
        scalar1=23, scalar2=None, op0=shl,
    )
    ve.tensor_scalar_add(t["sc"][:], t["sc"][:], 0x3F800000)
    return ve.tensor_tensor(
        t["decay"][:], t["q"][:], t["sc"][:].bitcast(dt), mult
    )


def _build_nc():
    if "nc" in _NC_CACHE:
        return _NC_CACHE["nc"]

    import concourse.bacc as bacc
    import concourse.mybir as mybir
    from concourse import tile

    dt = mybir.dt.float32
    nc = bacc.Bacc(
        "TRN2", target_bir_lowering=False, debug=False, num_devices=N_CORES
    )
    mem = nc.dram_tensor("mem", [RPC, DIM], dt, kind="ExternalInput")
    msg = nc.dram_tensor("msg", [RPC, DIM], dt, kind="ExternalInput")
    lu = nc.dram_tensor("lu", [RPC], dt, kind="ExternalInput")
    ts = nc.dram_tensor("ts", [RPC], dt, kind="ExternalInput")
    out = nc.dram_tensor("out", [RPC, DIM], dt, kind="ExternalOutput")

    # Partition p owns rows [p*NBLK, (p+1)*NBLK): per-partition DRAM runs are
    # contiguous (NBLK rows * 1 KB), so chunked DMAs move CH KB contiguous
    # per partition.
    mem_v = mem[:].rearrange("(p n) d -> p n d", p=P)
    msg_v = msg[:].rearrange("(p n) d -> p n d", p=P)
    out_v = out[:].rearrange("(p n) d -> p n d", p=P)
    lu_v = lu[:].rearrange("(p n) -> p n", p=P)
    ts_v = ts[:].rearrange("(p n) -> p n", p=P)

    mult = mybir.AluOpType.mult
    add = mybir.AluOpType.add

    with tile.TileContext(nc) as tc:
        with (
            tc.tile_pool(name="vec", bufs=1) as vpool,
            tc.tile_pool(name="memp", bufs=5) as mpool,
            tc.tile_pool(name="msgp", bufs=5) as spool,
        ):
            lu_t = vpool.tile([P, NBLK], dt)
            ts_t = vpool.tile([P, NBLK], dt)
            tiles = {
                n: vpool.tile([P, NBLK], dt, name=f"dc_{n}", tag=f"dc_{n}")
                for n in ("x", "y", "k", "u", "q", "decay")
            }
            tiles["sc"] = vpool.tile(
                [P, NBLK], mybir.dt.int32, name="dc_sc", tag="dc_sc"
            )
            decay_t = tiles["decay"]
            # lu/ts ride the ACT ring (idle at kernel start) so the first
            # big loads on the SP ring are not queued behind them.
            nc.scalar.dma_start(out=lu_t[:], in_=lu_v)
            nc.scalar.dma_start(out=ts_t[:], in_=ts_v)
            _emit_decay_chain(nc.vector, mybir, tiles, lu_t[:], ts_t[:])

            base = 0
            for ch in CHUNKS:
                mem_t = mpool.tile([P, ch, DIM], dt, tag="memt")
                msg_t = spool.tile([P, ch, DIM], dt, tag="msgt")
                nc.sync.dma_start(out=mem_t[:], in_=mem_v[:, base:base + ch, :])
                nc.sync.dma_start(out=msg_t[:], in_=msg_v[:, base:base + ch, :])
                # out = msg + decay*mem in two whole-chunk DVE ops; the decay
                # column block broadcasts along dim via a 0-stride AP.
                dec_b = decay_t[:, base:base + ch].broadcast_to([P, ch, DIM])
                nc.vector.tensor_tensor(mem_t[:], mem_t[:], dec_b, mult)
                nc.vector.tensor_tensor(msg_t[:], mem_t[:], msg_t[:], add)
                # Stores go on the ACT HWDGE ring so a store waiting on DVE
                # can't head-of-line-block later loads on the SP ring.
                nc.scalar.dma_start(out=out_v[:, base:base + ch, :], in_=msg_t[:])
                base += ch

    nc.finalize()
    _NC_CACHE["nc"] = nc
    return nc


def _build_nc_raw():
    """Raw-Bass (no Tile) build: manual semaphores, minimal preamble/drain.

    Engine roles: Sync issues loads (SP HWDGE ring), Vector computes,
    Scalar/ACT issues stores (ACT HWDGE ring). Buffer recycling is gated
    by cp_sem (compute done -> mem buf free) and st_sem (store done ->
    msg buf free); HWDGE completions on one ring are FIFO, so cumulative
    semaphore waits identify chunk completions.
    """
    if "nc_raw" in _NC_CACHE:
        return _NC_CACHE["nc_raw"]

    from contextlib import ExitStack

    import concourse.bacc as bacc
    import concourse.mybir as mybir

    dt = mybir.dt.float32
    mult = mybir.AluOpType.mult
    add = mybir.AluOpType.add
    B = 4

    nc = bacc.Bacc(
        "TRN2", target_bir_lowering=False, debug=False, num_devices=N_CORES
    )
    mem = nc.dram_tensor("mem", [RPC, DIM], dt, kind="ExternalInput")
    msg = nc.dram_tensor("msg", [RPC, DIM], dt, kind="ExternalInput")
    lu = nc.dram_tensor("lu", [RPC], dt, kind="ExternalInput")
    ts = nc.dram_tensor("ts", [RPC], dt, kind="ExternalInput")
    out = nc.dram_tensor("out", [RPC, DIM], dt, kind="ExternalOutput")

    mem_v = mem[:].rearrange("(p n) d -> p n d", p=P)
    msg_v = msg[:].rearrange("(p n) d -> p n d", p=P)
    out_v = out[:].rearrange("(p n) d -> p n d", p=P)
    lu_v = lu[:].rearrange("(p n) -> p n", p=P)
    ts_v = ts[:].rearrange("(p n) -> p n", p=P)

    spans = []
    base = 0
    for ch in CHUNKS:
        spans.append((base, ch))
        base += ch

    with ExitStack() as stack:
        mem_bufs = [
            stack.enter_context(nc.sbuf_tensor(f"membuf{j}", [P, CH_MAX, DIM], dt))
            for j in range(B)
        ]
        msg_bufs = [
            stack.enter_context(nc.sbuf_tensor(f"msgbuf{j}", [P, CH_MAX, DIM], dt))
            for j in range(B)
        ]
        lu_t = stack.enter_context(nc.sbuf_tensor("lu_t", [P, NBLK], dt))
        ts_t = stack.enter_context(nc.sbuf_tensor("ts_t", [P, NBLK], dt))
        tiles = {
            n: stack.enter_context(nc.sbuf_tensor(f"dc_{n}", [P, NBLK], dt))
            for n in ("x", "y", "k", "u", "q", "decay")
        }
        tiles["sc"] = stack.enter_context(
            nc.sbuf_tensor("dc_sc", [P, NBLK], mybir.dt.int32)
        )
        decay_t = tiles["decay"]

        # Per-buffer-slot semaphore lanes: a lane only ever has one
        # outstanding DMA pair (buffer gating serializes reuse), so the
        # cumulative wait value exactly identifies that chunk's completion.
        # A single shared counting sem would race: with several DMAs in
        # flight, a lagging SDMA engine on chunk i can be masked by incs
        # from chunk i+1, releasing the waiter early.
        ld_sems = [
            stack.enter_context(nc.semaphore(f"ld_sem{j}")) for j in range(B)
        ]
        st_sems = [
            stack.enter_context(nc.semaphore(f"st_sem{j}")) for j in range(B)
        ]
        lu_sem = stack.enter_context(nc.semaphore("lu_sem"))
        cp_sem = stack.enter_context(nc.semaphore("cp_sem"))
        block = stack.enter_context(nc.Block())

        @block.sync
        def _(sync):
            sync.dma_start(out=lu_t[:], in_=lu_v).then_inc(lu_sem, 16)
            sync.dma_start(out=ts_t[:], in_=ts_v).then_inc(lu_sem, 16)
            for i, (b0, ch) in enumerate(spans):
                s = i % B
                if i >= B:
                    # mem buf free when chunk i-B's compute is done; msg buf
                    # free when its store (same lane) is done.
                    sync.wait_ge(cp_sem, (i - B) + 1)
                    sync.wait_ge(st_sems[s], (i // B) * 16)
                sync.dma_start(
                    out=mem_bufs[s][:, :ch, :],
                    in_=mem_v[:, b0:b0 + ch, :],
                ).then_inc(ld_sems[s], 16)
                sync.dma_start(
                    out=msg_bufs[s][:, :ch, :],
                    in_=msg_v[:, b0:b0 + ch, :],
                ).then_inc(ld_sems[s], 16)

        @block.vector
        def _(vector):
            vector.wait_ge(lu_sem, 32)
            _emit_decay_chain(nc.vector, mybir, tiles, lu_t[:], ts_t[:])
            for i, (b0, ch) in enumerate(spans):
                s = i % B
                vector.wait_ge(ld_sems[s], (i // B + 1) * 32)
                mem_b = mem_bufs[s][:, :ch, :]
                msg_b = msg_bufs[s][:, :ch, :]
                dec_b = decay_t[:, b0:b0 + ch].broadcast_to([P, ch, DIM])
                nc.vector.tensor_tensor(mem_b, mem_b, dec_b, mult)
                nc.vector.tensor_tensor(msg_b, mem_b, msg_b, add).then_inc(
                    cp_sem, 1
                )

        @block.scalar
        def _(scalar):
            for i, (b0, ch) in enumerate(spans):
                s = i % B
                scalar.wait_ge(cp_sem, i + 1)
                scalar.dma_start(
                    out=out_v[:, b0:b0 + ch, :],
                    in_=msg_bufs[s][:, :ch, :],
                ).then_inc(st_sems[s], 16)
            for j in range(B):
                n_stores = len([i for i in range(len(spans)) if i % B == j])
                scalar.wait_ge(st_sems[j], n_stores * 16)

    nc.finalize()
    _NC_CACHE["nc_raw"] = nc
    return nc


def _prep_in_maps(mem_rows, msgs, lu_rows, ts):
    """Pad the gathered update rows to 8*RPC and split per core."""
    n = mem_rows.shape[0]
    total = N_CORES * RPC
    mem_p = np.zeros((total, DIM), dtype=np.float32)
    msg_p = np.zeros((total, DIM), dtype=np.float32)
    lu_p = np.zeros(total, dtype=np.float32)
    ts_p = np.zeros(total, dtype=np.float32)
    mem_p[:n] = mem_rows
    msg_p[:n] = msgs
    lu_p[:n] = lu_rows
    ts_p[:n] = ts
    return [
        {
            "mem": mem_p[c * RPC:(c + 1) * RPC],
            "msg": msg_p[c * RPC:(c + 1) * RPC],
            "lu": lu_p[c * RPC:(c + 1) * RPC],
            "ts": ts_p[c * RPC:(c + 1) * RPC],
        }
        for c in range(N_CORES)
    ]


def _run_device(in_maps, trace=False):
    import os

    _install_ntff_hook_shim()
    from concourse.bass_utils import run_bass_kernel_spmd

    # The raw-bass build is ~1us faster but intermittently hard-faults the
    # exec unit (NRT_EXEC_UNIT_UNRECOVERABLE); keep the Tile-scheduled build
    # as the default.
    nc = _build_nc_raw() if os.environ.get("EMU_RAW", "0") == "1" else _build_nc()
    return run_bass_kernel_spmd(
        nc, in_maps, list(range(N_CORES)), trace=trace
    )


def _updated_rows(res):
    """Concatenate per-core device outputs and strip padding."""
    outs = [res.results[c]["out"] for c in range(N_CORES)]
    return np.concatenate(outs, axis=0)[:N_UPD]


def kernel(memory, last_update, unique_node_ids, unique_messages, timestamps,
           _trace=False, _return_res=False):
    memory = np.asarray(memory)
    last_update = np.asarray(last_update)
    ids = np.asarray(unique_node_ids)
    msgs = np.asarray(unique_messages, dtype=np.float32)
    ts = np.asarray(timestamps, dtype=np.float32)
    n = ids.shape[0]

    contiguous = n == N_UPD and ids[0] == 0 and ids[-1] == n - 1 and np.array_equal(
        ids, np.arange(n, dtype=ids.dtype)
    )

    if contiguous:
        mem_rows = memory[:n]
        lu_rows = last_update[:n]
    else:
        mem_rows = memory[ids]
        lu_rows = last_update[ids]

    in_maps = _prep_in_maps(mem_rows, msgs, lu_rows, ts)
    res = _run_device(in_maps, trace=_trace)
    updated = _updated_rows(res)

    updated_memory = memory.copy()
    new_last_update = last_update.copy()
    if contiguous:
        updated_memory[:n] = updated
        new_last_update[:n] = ts
    else:
        updated_memory[ids] = updated
        new_last_update[ids] = ts

    if _return_res:
        return (updated_memory, new_last_update), res
    return updated_memory, new_last_update


# revision 31
# speedup vs baseline: 1.2521x; 1.2521x over previous
"""Trainium2 Bass kernel for ExpMemoryUpdater (scatter_memory).

Semantics (reference):
    mem_rows   = memory[unique_node_ids]                  # [n_upd, dim]
    decay      = exp((last_update[unique_node_ids] - timestamps) / LAMB)
    updated    = unique_messages + decay[:, None] * mem_rows
    updated_memory  = memory.at[unique_node_ids].set(updated)
    new_last_update = last_update.at[unique_node_ids].set(timestamps)

Distribution: the 100000 updated rows are row-sharded across 8 NeuronCores
(12500 rows each, padded to 12544 = 128*98). Each core computes its updated
rows; rows not touched by any update and the last_update scatter are pure
data placement, handled during host-side unshard/assembly.

On-core layout: partition p owns 98 consecutive rows of its shard, so every
DMA moves long contiguous per-partition runs (up to 14 KB per partition per
chunk, 1.75 MB per dma_start; ~368 GB/s effective, at the per-core HBM
roofline). decay = exp((lu-ts)/LAMB) is computed once as a [128, 98] tile
with a ~2 ulp software exp (Cody-Waite + Taylor; the ACT table Exp is only
~1e-5 accurate). The row update is two whole-chunk DVE ops, with decay
broadcast along dim via a 0-stride AP:
    mem *= decay ; out = mem + msg
Loads ride the SP HWDGE ring and stores the ACT ring, so a store waiting on
compute never head-of-line-blocks later loads. Final chunks shrink so the
compute+store tail past the last load is short. Measured ~104 us HW exec
vs a ~100 us DMA roofline (36.75 MB/core at 358 GB/s).
"""

import sys
import types

import numpy as np

N_CORES = 8
N_NODES = 200000
DIM = 256
N_UPD = 100000
LAMB = 10.0

P = 128          # SBUF partitions
NBLK = 98        # row-blocks per partition
RPC = P * NBLK   # rows per core, padded (12544)
# Row-blocks per DMA chunk. Large chunks amortize DMA setup; the final
# chunks shrink so the compute+store tail past the last load is short.
CHUNKS = [14, 14, 14, 14, 14, 14, 8, 4, 1, 1]
assert sum(CHUNKS) == NBLK
CH_MAX = max(CHUNKS)


def _install_ntff_hook_shim():
    """Make run_bass_kernel_spmd(trace=True) work under axon: the stock
    antenv package lacks axon_hooks, so register the ctypes NTFF hook from
    trn_agent_boot under that name."""
    try:
        import antenv.axon_hooks  # noqa: F401
        return
    except ImportError:
        pass
    try:
        import antenv
        from trn_agent_boot.trn_boot import _ntff_profile_via_ctypes

        mod = types.ModuleType("antenv.axon_hooks")
        hook = [_ntff_profile_via_ctypes("/opt/axon/libaxon_pjrt.so")]
        mod.set_axon_ntff_profile_hook = lambda h: hook.__setitem__(0, h)
        mod.get_axon_ntff_profile_hook = lambda: hook[0]
        sys.modules["antenv.axon_hooks"] = mod
        antenv.axon_hooks = mod
    except Exception:
        pass


_NC_CACHE = {}


def _emit_decay_chain(ve, mybir, tiles, lu_ap, ts_ap):
    """Emit the software-exp op chain on the vector engine `ve`.

    tiles: dict with f32 [P, NBLK] tiles lu,ts,x,y,k,u,q,decay and int32 sc.
    Computes decay = exp((lu - ts)/LAMB) to ~2 ulp.
    """
    import math

    sub = mybir.AluOpType.subtract
    mult = mybir.AluOpType.mult
    add = mybir.AluOpType.add
    shl = mybir.AluOpType.arith_shift_left
    dt = mybir.dt.float32

    INV_LAMB = float(np.float32(1.0 / LAMB))
    LOG2E = 1.4426950408889634
    MAGIC = 12582912.0  # 1.5 * 2^23
    LN2_HI = 0.693145751953125  # 0x3F317200, 9 trailing zero bits
    LN2_LO = 0.6931471805599453 - LN2_HI
    POLY = [1.0 / math.factorial(i) for i in range(8)]  # c0..c7

    t = tiles
    ve.tensor_tensor(t["x"][:], lu_ap, ts_ap, sub)
    ve.tensor_scalar_mul(t["x"][:], t["x"][:], INV_LAMB)
    # y = x*log2e + MAGIC;  k = y - MAGIC  (round-to-nearest)
    ve.tensor_scalar(
        out=t["y"][:], in0=t["x"][:], scalar1=LOG2E, scalar2=MAGIC,
        op0=mult, op1=add,
    )
    ve.tensor_scalar_sub(t["k"][:], t["y"][:], MAGIC)
    # u = (k*(-ln2_hi) + x) + k*(-ln2_lo)
    ve.scalar_tensor_tensor(
        out=t["u"][:], in0=t["k"][:], scalar=-LN2_HI, in1=t["x"][:],
        op0=mult, op1=add,
    )
    ve.scalar_tensor_tensor(
        out=t["u"][:], in0=t["k"][:], scalar=-LN2_LO, in1=t["u"][:],
        op0=mult, op1=add,
    )
    # exp(u) = c0 + u*(c1 + u*(... + u*c7)) via q = (q + c)*u steps
    ve.tensor_scalar_mul(t["q"][:], t["u"][:], POLY[7])
    for c_i in POLY[6:0:-1]:
        ve.scalar_tensor_tensor(
            out=t["q"][:], in0=t["q"][:], scalar=float(c_i), in1=t["u"][:],
            op0=add, op1=mult,
        )
    ve.tensor_scalar_add(t["q"][:], t["q"][:], 1.0)
    # bits(2^k) = (bits(y) << 23) + 0x3F800000
    ve.tensor_scalar(
        out=t["sc"][:], in0=t["y"][:].bitcast(mybir.dt.int32),
        scalar1=23, scalar2=None, op0=shl,
    )
    ve.tensor_scalar_add(t["sc"][:], t["sc"][:], 0x3F800000)
    return ve.tensor_tensor(
        t["decay"][:], t["q"][:], t["sc"][:].bitcast(dt), mult
    )


def _build_nc():
    if "nc" in _NC_CACHE:
        return _NC_CACHE["nc"]

    import concourse.bacc as bacc
    import concourse.mybir as mybir
    from concourse import tile

    dt = mybir.dt.float32
    nc = bacc.Bacc(
        "TRN2", target_bir_lowering=False, debug=False, num_devices=N_CORES
    )
    mem = nc.dram_tensor("mem", [RPC, DIM], dt, kind="ExternalInput")
    msg = nc.dram_tensor("msg", [RPC, DIM], dt, kind="ExternalInput")
    lu = nc.dram_tensor("lu", [RPC], dt, kind="ExternalInput")
    ts = nc.dram_tensor("ts", [RPC], dt, kind="ExternalInput")
    out = nc.dram_tensor("out", [RPC, DIM], dt, kind="ExternalOutput")

    # Partition p owns rows [p*NBLK, (p+1)*NBLK): per-partition DRAM runs are
    # contiguous (NBLK rows * 1 KB), so chunked DMAs move CH KB contiguous
    # per partition.
    mem_v = mem[:].rearrange("(p n) d -> p n d", p=P)
    msg_v = msg[:].rearrange("(p n) d -> p n d", p=P)
    out_v = out[:].rearrange("(p n) d -> p n d", p=P)
    lu_v = lu[:].rearrange("(p n) -> p n", p=P)
    ts_v = ts[:].rearrange("(p n) -> p n", p=P)

    mult = mybir.AluOpType.mult
    add = mybir.AluOpType.add

    with tile.TileContext(nc) as tc:
        with (
            tc.tile_pool(name="vec", bufs=1) as vpool,
            tc.tile_pool(name="memp", bufs=5) as mpool,
            tc.tile_pool(name="msgp", bufs=5) as spool,
        ):
            lu_t = vpool.tile([P, NBLK], dt)
            ts_t = vpool.tile([P, NBLK], dt)
            tiles = {
                n: vpool.tile([P, NBLK], dt, name=f"dc_{n}", tag=f"dc_{n}")
                for n in ("x", "y", "k", "u", "q", "decay")
            }
            tiles["sc"] = vpool.tile(
                [P, NBLK], mybir.dt.int32, name="dc_sc", tag="dc_sc"
            )
            decay_t = tiles["decay"]
            # lu/ts ride the ACT ring (idle at kernel start) so the first
            # big loads on the SP ring are not queued behind them.
            nc.scalar.dma_start(out=lu_t[:], in_=lu_v)
            nc.scalar.dma_start(out=ts_t[:], in_=ts_v)
            _emit_decay_chain(nc.vector, mybir, tiles, lu_t[:], ts_t[:])

            base = 0
            for ch in CHUNKS:
                mem_t = mpool.tile([P, ch, DIM], dt, tag="memt")
                msg_t = spool.tile([P, ch, DIM], dt, tag="msgt")
                nc.sync.dma_start(out=mem_t[:], in_=mem_v[:, base:base + ch, :])
                nc.sync.dma_start(out=msg_t[:], in_=msg_v[:, base:base + ch, :])
                # out = msg + decay*mem in two whole-chunk DVE ops; the decay
                # column block broadcasts along dim via a 0-stride AP.
                dec_b = decay_t[:, base:base + ch].broadcast_to([P, ch, DIM])
                nc.vector.tensor_tensor(mem_t[:], mem_t[:], dec_b, mult)
                nc.vector.tensor_tensor(msg_t[:], mem_t[:], msg_t[:], add)
                # Stores go on the ACT HWDGE ring so a store waiting on DVE
                # can't head-of-line-block later loads on the SP ring.
                nc.scalar.dma_start(out=out_v[:, base:base + ch, :], in_=msg_t[:])
                base += ch

    nc.finalize()
    _NC_CACHE["nc"] = nc
    return nc


def _build_nc_raw():
    """Raw-Bass (no Tile) build: manual semaphores, minimal preamble/drain.

    Engine roles: Sync issues loads (SP HWDGE ring), Vector computes,
    Scalar/ACT issues stores (ACT HWDGE ring). Buffer recycling is gated
    by cp_sem (compute done -> mem buf free) and st_sem (store done ->
    msg buf free); HWDGE completions on one ring are FIFO, so cumulative
    semaphore waits identify chunk completions.
    """
    if "nc_raw" in _NC_CACHE:
        return _NC_CACHE["nc_raw"]

    from contextlib import ExitStack

    import concourse.bacc as bacc
    import concourse.mybir as mybir

    dt = mybir.dt.float32
    mult = mybir.AluOpType.mult
    add = mybir.AluOpType.add
    B = 4

    nc = bacc.Bacc(
        "TRN2", target_bir_lowering=False, debug=False, num_devices=N_CORES
    )
    mem = nc.dram_tensor("mem", [RPC, DIM], dt, kind="ExternalInput")
    msg = nc.dram_tensor("msg", [RPC, DIM], dt, kind="ExternalInput")
    lu = nc.dram_tensor("lu", [RPC], dt, kind="ExternalInput")
    ts = nc.dram_tensor("ts", [RPC], dt, kind="ExternalInput")
    out = nc.dram_tensor("out", [RPC, DIM], dt, kind="ExternalOutput")

    mem_v = mem[:].rearrange("(p n) d -> p n d", p=P)
    msg_v = msg[:].rearrange("(p n) d -> p n d", p=P)
    out_v = out[:].rearrange("(p n) d -> p n d", p=P)
    lu_v = lu[:].rearrange("(p n) -> p n", p=P)
    ts_v = ts[:].rearrange("(p n) -> p n", p=P)

    spans = []
    base = 0
    for ch in CHUNKS:
        spans.append((base, ch))
        base += ch

    with ExitStack() as stack:
        mem_bufs = [
            stack.enter_context(nc.sbuf_tensor(f"membuf{j}", [P, CH_MAX, DIM], dt))
            for j in range(B)
        ]
        msg_bufs = [
            stack.enter_context(nc.sbuf_tensor(f"msgbuf{j}", [P, CH_MAX, DIM], dt))
            for j in range(B)
        ]
        lu_t = stack.enter_context(nc.sbuf_tensor("lu_t", [P, NBLK], dt))
        ts_t = stack.enter_context(nc.sbuf_tensor("ts_t", [P, NBLK], dt))
        tiles = {
            n: stack.enter_context(nc.sbuf_tensor(f"dc_{n}", [P, NBLK], dt))
            for n in ("x", "y", "k", "u", "q", "decay")
        }
        tiles["sc"] = stack.enter_context(
            nc.sbuf_tensor("dc_sc", [P, NBLK], mybir.dt.int32)
        )
        decay_t = tiles["decay"]

        # Per-buffer-slot semaphore lanes: a lane only ever has one
        # outstanding DMA pair (buffer gating serializes reuse), so the
        # cumulative wait value exactly identifies that chunk's completion.
        # A single shared counting sem would race: with several DMAs in
        # flight, a lagging SDMA engine on chunk i can be masked by incs
        # from chunk i+1, releasing the waiter early.
        ld_sems = [
            stack.enter_context(nc.semaphore(f"ld_sem{j}")) for j in range(B)
        ]
        st_sems = [
            stack.enter_context(nc.semaphore(f"st_sem{j}")) for j in range(B)
        ]
        lu_sem = stack.enter_context(nc.semaphore("lu_sem"))
        cp_sem = stack.enter_context(nc.semaphore("cp_sem"))
        block = stack.enter_context(nc.Block())

        @block.sync
        def _(sync):
            sync.dma_start(out=lu_t[:], in_=lu_v).then_inc(lu_sem, 16)
            sync.dma_start(out=ts_t[:], in_=ts_v).then_inc(lu_sem, 16)
            for i, (b0, ch) in enumerate(spans):
                s = i % B
                if i >= B:
                    # mem buf free when chunk i-B's compute is done; msg buf
                    # free when its store (same lane) is done.
                    sync.wait_ge(cp_sem, (i - B) + 1)
                    sync.wait_ge(st_sems[s], (i // B) * 16)
                sync.dma_start(
                    out=mem_bufs[s][:, :ch, :],
                    in_=mem_v[:, b0:b0 + ch, :],
                ).then_inc(ld_sems[s], 16)
                sync.dma_start(
                    out=msg_bufs[s][:, :ch, :],
                    in_=msg_v[:, b0:b0 + ch, :],
                ).then_inc(ld_sems[s], 16)

        @block.vector
        def _(vector):
            vector.wait_ge(lu_sem, 32)
            _emit_decay_chain(nc.vector, mybir, tiles, lu_t[:], ts_t[:])
            for i, (b0, ch) in enumerate(spans):
                s = i % B
                vector.wait_ge(ld_sems[s], (i // B + 1) * 32)
                mem_b = mem_bufs[s][:, :ch, :]
                msg_b = msg_bufs[s][:, :ch, :]
                dec_b = decay_t[:, b0:b0 + ch].broadcast_to([P, ch, DIM])
                nc.vector.tensor_tensor(mem_b, mem_b, dec_b, mult)
                nc.vector.tensor_tensor(msg_b, mem_b, msg_b, add).then_inc(
                    cp_sem, 1
                )

        @block.scalar
        def _(scalar):
            for i, (b0, ch) in enumerate(spans):
                s = i % B
                scalar.wait_ge(cp_sem, i + 1)
                scalar.dma_start(
                    out=out_v[:, b0:b0 + ch, :],
                    in_=msg_bufs[s][:, :ch, :],
                ).then_inc(st_sems[s], 16)
            for j in range(B):
                n_stores = len([i for i in range(len(spans)) if i % B == j])
                scalar.wait_ge(st_sems[j], n_stores * 16)

    nc.finalize()
    _NC_CACHE["nc_raw"] = nc
    return nc


def _prep_in_maps(mem_rows, msgs, lu_rows, ts):
    """Pad the gathered update rows to 8*RPC and split per core."""
    n = mem_rows.shape[0]
    total = N_CORES * RPC
    mem_p = np.zeros((total, DIM), dtype=np.float32)
    msg_p = np.zeros((total, DIM), dtype=np.float32)
    lu_p = np.zeros(total, dtype=np.float32)
    ts_p = np.zeros(total, dtype=np.float32)
    mem_p[:n] = mem_rows
    msg_p[:n] = msgs
    lu_p[:n] = lu_rows
    ts_p[:n] = ts
    return [
        {
            "mem": mem_p[c * RPC:(c + 1) * RPC],
            "msg": msg_p[c * RPC:(c + 1) * RPC],
            "lu": lu_p[c * RPC:(c + 1) * RPC],
            "ts": ts_p[c * RPC:(c + 1) * RPC],
        }
        for c in range(N_CORES)
    ]


def _run_device(in_maps, trace=False):
    import os

    _install_ntff_hook_shim()
    from concourse.bass_utils import run_bass_kernel_spmd

    # The raw-bass build is ~1us faster but intermittently hard-faults the
    # exec unit (NRT_EXEC_UNIT_UNRECOVERABLE); keep the Tile-scheduled build
    # as the default.
    nc = _build_nc_raw() if os.environ.get("EMU_RAW", "0") == "1" else _build_nc()
    return run_bass_kernel_spmd(
        nc, in_maps, list(range(N_CORES)), trace=trace
    )


def _updated_rows(res):
    """Concatenate per-core device outputs and strip padding."""
    outs = [res.results[c]["out"] for c in range(N_CORES)]
    return np.concatenate(outs, axis=0)[:N_UPD]


def kernel(memory, last_update, unique_node_ids, unique_messages, timestamps,
           _trace=False, _return_res=False):
    memory = np.asarray(memory)
    last_update = np.asarray(last_update)
    ids = np.asarray(unique_node_ids)
    msgs = np.asarray(unique_messages, dtype=np.float32)
    ts = np.asarray(timestamps, dtype=np.float32)
    n = ids.shape[0]

    contiguous = n == N_UPD and ids[0] == 0 and ids[-1] == n - 1 and np.array_equal(
        ids, np.arange(n, dtype=ids.dtype)
    )

    if contiguous:
        mem_rows = memory[:n]
        lu_rows = last_update[:n]
    else:
        mem_rows = memory[ids]
        lu_rows = last_update[ids]

    in_maps = _prep_in_maps(mem_rows, msgs, lu_rows, ts)
    res = _run_device(in_maps, trace=_trace)
    updated = _updated_rows(res)

    updated_memory = memory.copy()
    new_last_update = last_update.copy()
    if contiguous:
        updated_memory[:n] = updated
        new_last_update[:n] = ts
    else:
        updated_memory[ids] = updated
        new_last_update[ids] = ts

    if _return_res:
        return (updated_memory, new_last_update), res
    return updated_memory, new_last_update


# revision 32
# speedup vs baseline: 1.2830x; 1.0247x over previous
"""Trainium2 Bass kernel for ExpMemoryUpdater (scatter_memory).

Semantics (reference):
    mem_rows   = memory[unique_node_ids]                  # [n_upd, dim]
    decay      = exp((last_update[unique_node_ids] - timestamps) / LAMB)
    updated    = unique_messages + decay[:, None] * mem_rows
    updated_memory  = memory.at[unique_node_ids].set(updated)
    new_last_update = last_update.at[unique_node_ids].set(timestamps)

Distribution: the 100000 updated rows are row-sharded across 8 NeuronCores
(12500 rows each, padded to 12544 = 128*98). Each core computes its updated
rows; rows not touched by any update and the last_update scatter are pure
data placement, handled during host-side unshard/assembly.

On-core layout: partition p owns 98 consecutive rows of its shard, so every
DMA moves long contiguous per-partition runs (up to 14 KB per partition per
chunk, 1.75 MB per dma_start; ~368 GB/s effective, at the per-core HBM
roofline). decay = exp((lu-ts)/LAMB) is computed once as a [128, 98] tile
with a ~2 ulp software exp (Cody-Waite + Taylor; the ACT table Exp is only
~1e-5 accurate). The row update is two whole-chunk DVE ops, with decay
broadcast along dim via a 0-stride AP:
    mem *= decay ; out = mem + msg
Loads ride the SP HWDGE ring and stores the ACT ring, so a store waiting on
compute never head-of-line-blocks later loads. Final chunks shrink so the
compute+store tail past the last load is short. Measured ~104 us HW exec
vs a ~100 us DMA roofline (36.75 MB/core at 358 GB/s).
"""

import sys
import types

import numpy as np

N_CORES = 8
N_NODES = 200000
DIM = 256
N_UPD = 100000
LAMB = 10.0

P = 128          # SBUF partitions
NBLK = 98        # row-blocks per partition
RPC = P * NBLK   # rows per core, padded (12544)
# Row-blocks per DMA chunk. Large chunks amortize DMA setup; the final
# chunks shrink so the compute+store tail past the last load is short.
CHUNKS = [14, 14, 14, 14, 14, 14, 8, 4, 1, 1]
assert sum(CHUNKS) == NBLK
CH_MAX = max(CHUNKS)


def _install_ntff_hook_shim():
    """Make run_bass_kernel_spmd(trace=True) work under axon: the stock
    antenv package lacks axon_hooks, so register the ctypes NTFF hook from
    trn_agent_boot under that name."""
    try:
        import antenv.axon_hooks  # noqa: F401
        return
    except ImportError:
        pass
    try:
        import antenv
        from trn_agent_boot.trn_boot import _ntff_profile_via_ctypes

        mod = types.ModuleType("antenv.axon_hooks")
        hook = [_ntff_profile_via_ctypes("/opt/axon/libaxon_pjrt.so")]
        mod.set_axon_ntff_profile_hook = lambda h: hook.__setitem__(0, h)
        mod.get_axon_ntff_profile_hook = lambda: hook[0]
        sys.modules["antenv.axon_hooks"] = mod
        antenv.axon_hooks = mod
    except Exception:
        pass


_NC_CACHE = {}


def _emit_decay_chain(ve, mybir, tiles, lu_ap, ts_ap):
    """Emit the software-exp op chain on the vector engine `ve`.

    tiles: dict with f32 [P, NBLK] tiles lu,ts,x,y,k,u,q,decay and int32 sc.
    Computes decay = exp((lu - ts)/LAMB) to ~2 ulp.
    """
    import math

    sub = mybir.AluOpType.subtract
    mult = mybir.AluOpType.mult
    add = mybir.AluOpType.add
    shl = mybir.AluOpType.arith_shift_left
    dt = mybir.dt.float32

    INV_LAMB = float(np.float32(1.0 / LAMB))
    LOG2E = 1.4426950408889634
    MAGIC = 12582912.0  # 1.5 * 2^23
    LN2_HI = 0.693145751953125  # 0x3F317200, 9 trailing zero bits
    LN2_LO = 0.6931471805599453 - LN2_HI
    POLY = [1.0 / math.factorial(i) for i in range(8)]  # c0..c7

    t = tiles
    ve.tensor_tensor(t["x"][:], lu_ap, ts_ap, sub)
    ve.tensor_scalar_mul(t["x"][:], t["x"][:], INV_LAMB)
    # y = x*log2e + MAGIC;  k = y - MAGIC  (round-to-nearest)
    ve.tensor_scalar(
        out=t["y"][:], in0=t["x"][:], scalar1=LOG2E, scalar2=MAGIC,
        op0=mult, op1=add,
    )
    ve.tensor_scalar_sub(t["k"][:], t["y"][:], MAGIC)
    # u = (k*(-ln2_hi) + x) + k*(-ln2_lo)
    ve.scalar_tensor_tensor(
        out=t["u"][:], in0=t["k"][:], scalar=-LN2_HI, in1=t["x"][:],
        op0=mult, op1=add,
    )
    ve.scalar_tensor_tensor(
        out=t["u"][:], in0=t["k"][:], scalar=-LN2_LO, in1=t["u"][:],
        op0=mult, op1=add,
    )
    # exp(u) = c0 + u*(c1 + u*(... + u*c7)) via q = (q + c)*u steps
    ve.tensor_scalar_mul(t["q"][:], t["u"][:], POLY[7])
    for c_i in POLY[6:0:-1]:
        ve.scalar_tensor_tensor(
            out=t["q"][:], in0=t["q"][:], scalar=float(c_i), in1=t["u"][:],
            op0=add, op1=mult,
        )
    ve.tensor_scalar_add(t["q"][:], t["q"][:], 1.0)
    # bits(2^k) = (bits(y) << 23) + 0x3F800000
    ve.tensor_scalar(
        out=t["sc"][:], in0=t["y"][:].bitcast(mybir.dt.int32),
        scalar1=23, scalar2=None, op0=shl,
    )
    ve.tensor_scalar_add(t["sc"][:], t["sc"][:], 0x3F800000)
    return ve.tensor_tensor(
        t["decay"][:], t["q"][:], t["sc"][:].bitcast(dt), mult
    )


def _build_nc():
    if "nc" in _NC_CACHE:
        return _NC_CACHE["nc"]

    import concourse.bacc as bacc
    import concourse.mybir as mybir
    from concourse import tile

    dt = mybir.dt.float32
    nc = bacc.Bacc(
        "TRN2", target_bir_lowering=False, debug=False, num_devices=N_CORES
    )
    mem = nc.dram_tensor("mem", [RPC, DIM], dt, kind="ExternalInput")
    msg = nc.dram_tensor("msg", [RPC, DIM], dt, kind="ExternalInput")
    lu = nc.dram_tensor("lu", [RPC], dt, kind="ExternalInput")
    ts = nc.dram_tensor("ts", [RPC], dt, kind="ExternalInput")
    out = nc.dram_tensor("out", [RPC, DIM], dt, kind="ExternalOutput")

    # Partition p owns rows [p*NBLK, (p+1)*NBLK): per-partition DRAM runs are
    # contiguous (NBLK rows * 1 KB), so chunked DMAs move CH KB contiguous
    # per partition.
    mem_v = mem[:].rearrange("(p n) d -> p n d", p=P)
    msg_v = msg[:].rearrange("(p n) d -> p n d", p=P)
    out_v = out[:].rearrange("(p n) d -> p n d", p=P)
    lu_v = lu[:].rearrange("(p n) -> p n", p=P)
    ts_v = ts[:].rearrange("(p n) -> p n", p=P)

    mult = mybir.AluOpType.mult
    add = mybir.AluOpType.add

    with tile.TileContext(nc) as tc:
        with (
            tc.tile_pool(name="vec", bufs=1) as vpool,
            tc.tile_pool(name="memp", bufs=5) as mpool,
            tc.tile_pool(name="msgp", bufs=5) as spool,
        ):
            lu_t = vpool.tile([P, NBLK], dt)
            ts_t = vpool.tile([P, NBLK], dt)
            tiles = {
                n: vpool.tile([P, NBLK], dt, name=f"dc_{n}", tag=f"dc_{n}")
                for n in ("x", "y", "k", "u", "q", "decay")
            }
            tiles["sc"] = vpool.tile(
                [P, NBLK], mybir.dt.int32, name="dc_sc", tag="dc_sc"
            )
            decay_t = tiles["decay"]
            # lu/ts ride the ACT ring (idle at kernel start) so the first
            # big loads on the SP ring are not queued behind them.
            nc.scalar.dma_start(out=lu_t[:], in_=lu_v)
            nc.scalar.dma_start(out=ts_t[:], in_=ts_v)
            _emit_decay_chain(nc.vector, mybir, tiles, lu_t[:], ts_t[:])

            base = 0
            for ci, ch in enumerate(CHUNKS):
                mem_t = mpool.tile([P, ch, DIM], dt, tag="memt")
                msg_t = spool.tile([P, ch, DIM], dt, tag="msgt")
                # Chunk 0 loads ride the ACT ring: the Scalar engine clears
                # its preamble ~2.5us before Sync, so the first big loads
                # start during Sync's preamble instead of idling the SDMAs.
                ld = nc.scalar if ci == 0 else nc.sync
                ld.dma_start(out=mem_t[:], in_=mem_v[:, base:base + ch, :])
                ld.dma_start(out=msg_t[:], in_=msg_v[:, base:base + ch, :])
                # out = msg + decay*mem in two whole-chunk DVE ops; the decay
                # column block broadcasts along dim via a 0-stride AP.
                dec_b = decay_t[:, base:base + ch].broadcast_to([P, ch, DIM])
                nc.vector.tensor_tensor(mem_t[:], mem_t[:], dec_b, mult)
                nc.vector.tensor_tensor(msg_t[:], mem_t[:], msg_t[:], add)
                # Stores go on the ACT HWDGE ring so a store waiting on DVE
                # can't head-of-line-block later loads on the SP ring.
                nc.scalar.dma_start(out=out_v[:, base:base + ch, :], in_=msg_t[:])
                base += ch

    nc.finalize()
    _NC_CACHE["nc"] = nc
    return nc


def _build_nc_raw():
    """Raw-Bass (no Tile) build: manual semaphores, minimal preamble/drain.

    Engine roles: Sync issues loads (SP HWDGE ring), Vector computes,
    Scalar/ACT issues stores (ACT HWDGE ring). Buffer recycling is gated
    by cp_sem (compute done -> mem buf free) and st_sem (store done ->
    msg buf free); HWDGE completions on one ring are FIFO, so cumulative
    semaphore waits identify chunk completions.
    """
    if "nc_raw" in _NC_CACHE:
        return _NC_CACHE["nc_raw"]

    from contextlib import ExitStack

    import concourse.bacc as bacc
    import concourse.mybir as mybir

    dt = mybir.dt.float32
    mult = mybir.AluOpType.mult
    add = mybir.AluOpType.add
    B = 4

    nc = bacc.Bacc(
        "TRN2", target_bir_lowering=False, debug=False, num_devices=N_CORES
    )
    mem = nc.dram_tensor("mem", [RPC, DIM], dt, kind="ExternalInput")
    msg = nc.dram_tensor("msg", [RPC, DIM], dt, kind="ExternalInput")
    lu = nc.dram_tensor("lu", [RPC], dt, kind="ExternalInput")
    ts = nc.dram_tensor("ts", [RPC], dt, kind="ExternalInput")
    out = nc.dram_tensor("out", [RPC, DIM], dt, kind="ExternalOutput")

    mem_v = mem[:].rearrange("(p n) d -> p n d", p=P)
    msg_v = msg[:].rearrange("(p n) d -> p n d", p=P)
    out_v = out[:].rearrange("(p n) d -> p n d", p=P)
    lu_v = lu[:].rearrange("(p n) -> p n", p=P)
    ts_v = ts[:].rearrange("(p n) -> p n", p=P)

    spans = []
    base = 0
    for ch in CHUNKS:
        spans.append((base, ch))
        base += ch

    with ExitStack() as stack:
        mem_bufs = [
            stack.enter_context(nc.sbuf_tensor(f"membuf{j}", [P, CH_MAX, DIM], dt))
            for j in range(B)
        ]
        msg_bufs = [
            stack.enter_context(nc.sbuf_tensor(f"msgbuf{j}", [P, CH_MAX, DIM], dt))
            for j in range(B)
        ]
        lu_t = stack.enter_context(nc.sbuf_tensor("lu_t", [P, NBLK], dt))
        ts_t = stack.enter_context(nc.sbuf_tensor("ts_t", [P, NBLK], dt))
        tiles = {
            n: stack.enter_context(nc.sbuf_tensor(f"dc_{n}", [P, NBLK], dt))
            for n in ("x", "y", "k", "u", "q", "decay")
        }
        tiles["sc"] = stack.enter_context(
            nc.sbuf_tensor("dc_sc", [P, NBLK], mybir.dt.int32)
        )
        decay_t = tiles["decay"]

        # Per-buffer-slot semaphore lanes: a lane only ever has one
        # outstanding DMA pair (buffer gating serializes reuse), so the
        # cumulative wait value exactly identifies that chunk's completion.
        # A single shared counting sem would race: with several DMAs in
        # flight, a lagging SDMA engine on chunk i can be masked by incs
        # from chunk i+1, releasing the waiter early.
        ld_sems = [
            stack.enter_context(nc.semaphore(f"ld_sem{j}")) for j in range(B)
        ]
        st_sems = [
            stack.enter_context(nc.semaphore(f"st_sem{j}")) for j in range(B)
        ]
        lu_sem = stack.enter_context(nc.semaphore("lu_sem"))
        cp_sem = stack.enter_context(nc.semaphore("cp_sem"))
        block = stack.enter_context(nc.Block())

        @block.sync
        def _(sync):
            sync.dma_start(out=lu_t[:], in_=lu_v).then_inc(lu_sem, 16)
            sync.dma_start(out=ts_t[:], in_=ts_v).then_inc(lu_sem, 16)
            for i, (b0, ch) in enumerate(spans):
                s = i % B
                if i >= B:
                    # mem buf free when chunk i-B's compute is done; msg buf
                    # free when its store (same lane) is done.
                    sync.wait_ge(cp_sem, (i - B) + 1)
                    sync.wait_ge(st_sems[s], (i // B) * 16)
                sync.dma_start(
                    out=mem_bufs[s][:, :ch, :],
                    in_=mem_v[:, b0:b0 + ch, :],
                ).then_inc(ld_sems[s], 16)
                sync.dma_start(
                    out=msg_bufs[s][:, :ch, :],
                    in_=msg_v[:, b0:b0 + ch, :],
                ).then_inc(ld_sems[s], 16)

        @block.vector
        def _(vector):
            vector.wait_ge(lu_sem, 32)
            _emit_decay_chain(nc.vector, mybir, tiles, lu_t[:], ts_t[:])
            for i, (b0, ch) in enumerate(spans):
                s = i % B
                vector.wait_ge(ld_sems[s], (i // B + 1) * 32)
                mem_b = mem_bufs[s][:, :ch, :]
                msg_b = msg_bufs[s][:, :ch, :]
                dec_b = decay_t[:, b0:b0 + ch].broadcast_to([P, ch, DIM])
                nc.vector.tensor_tensor(mem_b, mem_b, dec_b, mult)
                nc.vector.tensor_tensor(msg_b, mem_b, msg_b, add).then_inc(
                    cp_sem, 1
                )

        @block.scalar
        def _(scalar):
            for i, (b0, ch) in enumerate(spans):
                s = i % B
                scalar.wait_ge(cp_sem, i + 1)
                scalar.dma_start(
                    out=out_v[:, b0:b0 + ch, :],
                    in_=msg_bufs[s][:, :ch, :],
                ).then_inc(st_sems[s], 16)
            for j in range(B):
                n_stores = len([i for i in range(len(spans)) if i % B == j])
                scalar.wait_ge(st_sems[j], n_stores * 16)

    nc.finalize()
    _NC_CACHE["nc_raw"] = nc
    return nc


def _prep_in_maps(mem_rows, msgs, lu_rows, ts):
    """Pad the gathered update rows to 8*RPC and split per core."""
    n = mem_rows.shape[0]
    total = N_CORES * RPC
    mem_p = np.zeros((total, DIM), dtype=np.float32)
    msg_p = np.zeros((total, DIM), dtype=np.float32)
    lu_p = np.zeros(total, dtype=np.float32)
    ts_p = np.zeros(total, dtype=np.float32)
    mem_p[:n] = mem_rows
    msg_p[:n] = msgs
    lu_p[:n] = lu_rows
    ts_p[:n] = ts
    return [
        {
            "mem": mem_p[c * RPC:(c + 1) * RPC],
            "msg": msg_p[c * RPC:(c + 1) * RPC],
            "lu": lu_p[c * RPC:(c + 1) * RPC],
            "ts": ts_p[c * RPC:(c + 1) * RPC],
        }
        for c in range(N_CORES)
    ]


def _run_device(in_maps, trace=False):
    import os

    _install_ntff_hook_shim()
    from concourse.bass_utils import run_bass_kernel_spmd

    # The raw-bass build is ~1us faster but intermittently hard-faults the
    # exec unit (NRT_EXEC_UNIT_UNRECOVERABLE); keep the Tile-scheduled build
    # as the default.
    nc = _build_nc_raw() if os.environ.get("EMU_RAW", "0") == "1" else _build_nc()
    return run_bass_kernel_spmd(
        nc, in_maps, list(range(N_CORES)), trace=trace
    )


def _updated_rows(res):
    """Concatenate per-core device outputs and strip padding."""
    outs = [res.results[c]["out"] for c in range(N_CORES)]
    return np.concatenate(outs, axis=0)[:N_UPD]


def kernel(memory, last_update, unique_node_ids, unique_messages, timestamps,
           _trace=False, _return_res=False):
    memory = np.asarray(memory)
    last_update = np.asarray(last_update)
    ids = np.asarray(unique_node_ids)
    msgs = np.asarray(unique_messages, dtype=np.float32)
    ts = np.asarray(timestamps, dtype=np.float32)
    n = ids.shape[0]

    contiguous = n == N_UPD and ids[0] == 0 and ids[-1] == n - 1 and np.array_equal(
        ids, np.arange(n, dtype=ids.dtype)
    )

    if contiguous:
        mem_rows = memory[:n]
        lu_rows = last_update[:n]
    else:
        mem_rows = memory[ids]
        lu_rows = last_update[ids]

    in_maps = _prep_in_maps(mem_rows, msgs, lu_rows, ts)
    res = _run_device(in_maps, trace=_trace)
    updated = _updated_rows(res)

    updated_memory = memory.copy()
    new_last_update = last_update.copy()
    if contiguous:
        updated_memory[:n] = updated
        new_last_update[:n] = ts
    else:
        updated_memory[ids] = updated
        new_last_update[ids] = ts

    if _return_res:
        return (updated_memory, new_last_update), res
    return updated_memory, new_last_update
